# revision 1
# baseline (speedup 1.0000x reference)
"""Trainium2 Bass kernel for nn_CombinatorialClassifier (segment_reduce).

Strategy (8 NeuronCores, tensor-parallel over the num_partitionings axis):
  - Core i owns partitionings {2i, 2i+1}: a [2000, 2048] slice of W.
  - On device: logits = x @ Wshard.T (+ b row folded into the matmul),
    per-partitioning softmax -> probs [64, 2000], then a gpsimd ap_gather
    picks probs[b, idx(p, c)] for every class and the two partitionings are
    summed -> per-core partial [64, 50000].
  - Host: sum the 8 partials over cores (the all-reduce of the sharding
    hint), normalize over classes, log.

Gather layout: the probs tile is duplicated across SBUF partitions 0-63 /
64-127 so all 8 gpsimd Q7 cores work; quadrant A handles classes
[0, 25000), quadrant B [25000, 50000).
"""

import os
from contextlib import ExitStack

import numpy as np

import concourse.bacc as bacc
import concourse.mybir as mybir
import concourse.tile as tile
from concourse import bass_utils

B, P, K, C, D = 64, 16, 1000, 50000, 2048
ESP = 1e-20
NCORES = 8
PPC = P // NCORES        # partitionings per core
NLOC = PPC * K           # local logits width (2000)
NT = 500                 # matmul N-tile (PSUM bank: 500 fp32 <= 512)
NNT = NLOC // NT         # 4 N-tiles
DCH = D // 128           # 16 contraction chunks of 128
CQ = C // 2              # classes per gather quadrant stream (25000)
JC = 2048                # classes per gather call per quadrant

_chunks = []
_c = 0
while _c < CQ:
    _chunks.append(min(JC, CQ - _c))
    _c += JC
IDXCOLS = sum(2 * jc // 16 for jc in _chunks)  # int16 cols of gather indices

_F32 = mybir.dt.float32
_F16 = mybir.dt.float16
_I16 = mybir.dt.int16

_CACHE = {}
LAST_RESULTS = None


def _build_nc():
    nc = bacc.Bacc(
        "TRN2",
        target_bir_lowering=False,
        debug=False,
        enable_asserts=False,
        num_devices=NCORES,
    )
    xT_d = nc.dram_tensor("xT", [D, B], _F16, kind="ExternalInput")
    wtb_d = nc.dram_tensor("wtb", [D + 1, NLOC], _F16, kind="ExternalInput")
    idx_d = nc.dram_tensor("idx", [128, IDXCOLS], _I16, kind="ExternalInput")
    out_d = nc.dram_tensor("part_out", [B, C], _F32, kind="ExternalOutput")

    with tile.TileContext(nc) as tc, ExitStack() as ctx:
        const = ctx.enter_context(tc.tile_pool(name="const", bufs=1))
        wpool = ctx.enter_context(tc.tile_pool(name="w", bufs=3))
        spool = ctx.enter_context(tc.tile_pool(name="stats", bufs=1))
        gpool = ctx.enter_context(tc.tile_pool(name="g", bufs=3))
        apool = ctx.enter_context(tc.tile_pool(name="a", bufs=3))
        psum = ctx.enter_context(
            tc.tile_pool(name="psum", bufs=1, space="PSUM")
        )

        xt = const.tile([128, DCH, B], _F16)
        nc.sync.dma_start(xt[:], xT_d.ap().rearrange("(c p) b -> p c b", p=128))
        ones = const.tile([1, B], _F16)
        nc.vector.memset(ones[:], 1.0)
        bias = const.tile([1, NLOC], _F16)
        nc.sync.dma_start(bias[:], wtb_d[D : D + 1, :])
        idx_sb = const.tile([128, IDXCOLS], _I16)
        nc.sync.dma_start(idx_sb[:], idx_d.ap())
        gsrc = const.tile([128, NLOC], _F32)

        # ---- logits = x @ Wshard.T + b, accumulated in 4 PSUM banks ----
        ps = [psum.tile([B, NT], _F32, tag=f"ps{n}", name=f"ps{n}") for n in range(NNT)]
        for j in range(DCH):
            wt = wpool.tile([128, NLOC], _F16)
            nc.sync.dma_start(wt[:], wtb_d[128 * j : 128 * (j + 1), :])
            for n in range(NNT):
                nc.tensor.matmul(
                    ps[n][:],
                    xt[:, j, :],
                    wt[:, NT * n : NT * (n + 1)],
                    start=(j == 0),
                    stop=False,
                )
        for n in range(NNT):
            nc.tensor.matmul(
                ps[n][:],
                ones[:],
                bias[:, NT * n : NT * (n + 1)],
                start=False,
                stop=True,
            )

        # ---- per-partitioning softmax -> probs in gsrc[0:64] ----
        mx = spool.tile([B, NNT], _F32)
        for n in range(NNT):
            nc.vector.reduce_max(
                mx[:, n : n + 1], ps[n][:], axis=mybir.AxisListType.X
            )
        neg = spool.tile([B, PPC], _F32)
        for h in range(PPC):
            nc.vector.tensor_tensor(
                neg[:, h : h + 1],
                mx[:, 2 * h : 2 * h + 1],
                mx[:, 2 * h + 1 : 2 * h + 2],
                op=mybir.AluOpType.max,
            )
        nc.vector.tensor_scalar_mul(neg[:], neg[:], -1.0)
        sacc = spool.tile([B, NNT], _F32)
        for n in range(NNT):
            h = n // 2
            nc.scalar.activation(
                gsrc[0:B, NT * n : NT * (n + 1)],
                ps[n][:],
                mybir.ActivationFunctionType.Exp,
                bias=neg[:, h : h + 1],
                accum_out=sacc[:, n : n + 1],
            )
        rec = spool.tile([B, PPC], _F32)
        for h in range(PPC):
            nc.vector.tensor_tensor(
                rec[:, h : h + 1],
                sacc[:, 2 * h : 2 * h + 1],
                sacc[:, 2 * h + 1 : 2 * h + 2],
                op=mybir.AluOpType.add,
            )
        nc.vector.reciprocal(rec[:], rec[:])
        for h in range(PPC):
            nc.vector.tensor_scalar_mul(
                gsrc[0:B, K * h : K * (h + 1)],
                gsrc[0:B, K * h : K * (h + 1)],
                rec[:, h : h + 1],
            )
        # duplicate probs for the second gpsimd quadrant
        nc.sync.dma_start(gsrc[B : 2 * B, :], gsrc[0:B, :])

        # ---- gather + partitioning-sum + store ----
        offs = 0
        cum = 0
        for jc in _chunks:
            S = 2 * jc // 16
            g = gpool.tile([128, 2 * JC], _F32, tag="g")
            nc.gpsimd.ap_gather(
                g[:, 0 : 2 * jc],
                gsrc[:, :],
                idx_sb[:, offs : offs + S],
                channels=128,
                num_elems=NLOC,
                d=1,
                num_idxs=2 * jc,
            )
            a = apool.tile([128, JC], _F32, tag="a")
            nc.vector.tensor_add(a[:, 0:jc], g[:, 0:jc], g[:, jc : 2 * jc])
            nc.sync.dma_start(out_d[:, cum : cum + jc], a[0:B, 0:jc])
            nc.sync.dma_start(
                out_d[:, CQ + cum : CQ + cum + jc], a[B : 2 * B, 0:jc]
            )
            offs += S
            cum += jc

    nc.compile()
    return nc


def _host_inputs(x, W, b, part):
    """Per-core input maps: xT, wtb (W.T shard + bias row), gather indices."""
    xT = np.ascontiguousarray(x.T.astype(np.float16))
    part = np.asarray(part).astype(np.int64, copy=False)
    in_maps = []
    for i in range(NCORES):
        r0 = NLOC * i
        wtb = np.empty((D + 1, NLOC), np.float16)
        wtb[:D] = W[r0 : r0 + NLOC].T
        wtb[D] = b[r0 : r0 + NLOC]

        pa = part[2 * i] - (2 * i) * K            # [C] in [0, K)
        pb = part[2 * i + 1] - (2 * i + 1) * K + K  # [C] in [K, 2K)
        idxh = np.empty((128, IDXCOLS), np.int16)
        off = 0
        c0 = 0
        for jc in _chunks:
            S = 2 * jc // 16
            for q in range(2):
                base = q * CQ + c0
                L = np.concatenate(
                    [pa[base : base + jc], pb[base : base + jc]]
                ).astype(np.int16)
                blk = L.reshape(S, 16).T  # out col i <- (partition i%16, col i//16)
                for g4 in range(4):
                    p0 = q * 64 + g4 * 16
                    idxh[p0 : p0 + 16, off : off + S] = blk
            off += S
            c0 += jc
        in_maps.append({"xT": xT, "wtb": wtb, "idx": idxh})
    return in_maps


def kernel(**inputs):
    global LAST_RESULTS
    x = np.asarray(inputs["input"], dtype=np.float32)
    W = np.asarray(inputs["W"], dtype=np.float32)
    b = np.asarray(inputs["b"], dtype=np.float32)
    part = np.asarray(inputs["partitionings"])
    assert x.shape == (B, D) and W.shape == (P * K, D)

    if "nc" not in _CACHE:
        _CACHE["nc"] = _build_nc()
    nc = _CACHE["nc"]

    in_maps = _host_inputs(x, W, b, part)
    trace = bool(int(os.environ.get("BASSK_TRACE", "0")))
    res = bass_utils.run_bass_kernel_spmd(
        nc,
        in_maps,
        core_ids=list(range(NCORES)),
        trace=trace,
        tmpdir=os.environ.get("BASSK_TRACE_DIR") or None,
    )
    LAST_RESULTS = res

    acc = res.results[0]["part_out"].astype(np.float32)
    for i in range(1, NCORES):
        acc = acc + res.results[i]["part_out"]
    tot = acc.sum(axis=1, keepdims=True)
    return np.log(acc / tot + ESP).astype(np.float32)



# revision 2
# speedup vs baseline: 1.9198x; 1.9198x over previous
"""Trainium2 Bass kernel for nn_CombinatorialClassifier (segment_reduce).

Strategy (8 NeuronCores, tensor-parallel over the num_partitionings axis):
  - Core i owns partitionings {2i, 2i+1}: a [2000, 2048] slice of W.
  - On device: logits = x @ Wshard.T (+ b row folded into the matmul),
    per-partitioning softmax -> probs [64, 2000] (fp16).
  - Gather: probs are repacked into a batch-interleaved table
    TBL[ch, m, j] = probs[(ch%16) + 16*j, m] (fp16, d=4), replicated
    across all 8 Q7 core groups. One ap_gather index then fetches the
    probs for FOUR batch rows at once, so each Q7 core only processes
    2*6250 indices (its own 6250-class range, both partitionings)
    instead of 50000 -- ~3.3x less gpsimd command overhead than the
    d=1 layout.
  - The two partitionings are summed on the vector engine (with a free
    (class, j) -> (j, class) transpose fused in), then DMAed to DRAM as
    a fp16 [64, 50000] partial.
  - Host: sum the 8 partials over cores (the all-reduce of the sharding
    hint), normalize over classes, log.
"""

import os
from contextlib import ExitStack

import numpy as np

import concourse.bacc as bacc
import concourse.mybir as mybir
import concourse.tile as tile
from concourse import bass_utils
from concourse import library_config

B, P, K, C, D = 64, 16, 1000, 50000, 2048
ESP = 1e-20
NCORES = 8
PPC = P // NCORES        # partitionings per core (2)
NLOC = PPC * K           # local logits width (2000)
NT = 500                 # matmul N-tile (PSUM bank: 500 fp32 <= 512)
NNT = NLOC // NT         # 4 N-tiles
DCH = D // 128           # 16 contraction chunks of 128

NG = 8                   # Q7 core groups (16 partitions each)
CG = C // NG             # classes per group (6250)
NCH = 5                  # gather chunks per group
JCC = CG // NCH          # classes per chunk (1250)
NIDX = 2 * JCC           # gather indices per chunk (A then B) = 2500
ICOLS = ((NIDX + 31) // 32) * 2  # int16 idx cols per chunk (158)
ICT = NCH * ICOLS        # total idx cols (790)

_F32 = mybir.dt.float32
_F16 = mybir.dt.float16
_I16 = mybir.dt.int16

_CACHE = {}
LAST_RESULTS = None


def _build_nc():
    nc = bacc.Bacc(
        "TRN2",
        target_bir_lowering=False,
        debug=False,
        enable_asserts=False,
        num_devices=NCORES,
    )
    xT_d = nc.dram_tensor("xT", [D, B], _F16, kind="ExternalInput")
    wtb_d = nc.dram_tensor("wtb", [D + 1, NLOC], _F16, kind="ExternalInput")
    idx_d = nc.dram_tensor("idx", [128, ICT], _I16, kind="ExternalInput")
    out_d = nc.dram_tensor("part_out", [B, C], _F16, kind="ExternalOutput")

    with tile.TileContext(nc) as tc, ExitStack() as ctx:
        const = ctx.enter_context(tc.tile_pool(name="const", bufs=1))
        wpool = ctx.enter_context(tc.tile_pool(name="w", bufs=3))
        spool = ctx.enter_context(tc.tile_pool(name="stats", bufs=1))
        gpool = ctx.enter_context(tc.tile_pool(name="g", bufs=3))
        apool = ctx.enter_context(tc.tile_pool(name="a", bufs=3))
        psum = ctx.enter_context(
            tc.tile_pool(name="psum", bufs=1, space="PSUM")
        )

        xt = const.tile([128, DCH, B], _F16)
        nc.sync.dma_start(xt[:], xT_d.ap().rearrange("(c p) b -> p c b", p=128))
        ones = const.tile([1, B], _F16)
        nc.vector.memset(ones[:], 1.0)
        bias = const.tile([1, NLOC], _F16)
        nc.sync.dma_start(bias[:], wtb_d[D : D + 1, :])
        idx_sb = const.tile([128, ICT], _I16)
        nc.sync.dma_start(idx_sb[:], idx_d.ap())

        # get the gpsimd library load off the critical path: it can load
        # during the matmul instead of right before the first gather
        nc.gpsimd.load_library(library_config.ap_gather)

        # ---- logits = x @ Wshard.T + b, accumulated in 4 PSUM banks ----
        ps = [psum.tile([B, NT], _F32, tag=f"ps{n}", name=f"ps{n}") for n in range(NNT)]
        for j in range(DCH):
            wt = wpool.tile([128, NLOC], _F16)
            nc.sync.dma_start(wt[:], wtb_d[128 * j : 128 * (j + 1), :])
            for n in range(NNT):
                nc.tensor.matmul(
                    ps[n][:],
                    xt[:, j, :],
                    wt[:, NT * n : NT * (n + 1)],
                    start=(j == 0),
                    stop=False,
                )
        for n in range(NNT):
            nc.tensor.matmul(
                ps[n][:],
                ones[:],
                bias[:, NT * n : NT * (n + 1)],
                start=False,
                stop=True,
            )

        # ---- per-partitioning softmax -> probs16 [64, 2000] fp16 ----
        exp32 = const.tile([B, NLOC], _F32)
        probs16 = const.tile([B, NLOC], _F16)
        mx = spool.tile([B, NNT], _F32)
        for n in range(NNT):
            nc.vector.reduce_max(
                mx[:, n : n + 1], ps[n][:], axis=mybir.AxisListType.X
            )
        neg = spool.tile([B, PPC], _F32)
        for h in range(PPC):
            nc.vector.tensor_tensor(
                neg[:, h : h + 1],
                mx[:, 2 * h : 2 * h + 1],
                mx[:, 2 * h + 1 : 2 * h + 2],
                op=mybir.AluOpType.max,
            )
        nc.vector.tensor_scalar_mul(neg[:], neg[:], -1.0)
        sacc = spool.tile([B, NNT], _F32)
        for n in range(NNT):
            h = n // 2
            nc.scalar.activation(
                exp32[:, NT * n : NT * (n + 1)],
                ps[n][:],
                mybir.ActivationFunctionType.Exp,
                bias=neg[:, h : h + 1],
                accum_out=sacc[:, n : n + 1],
            )
        rec = spool.tile([B, PPC], _F32)
        for h in range(PPC):
            nc.vector.tensor_tensor(
                rec[:, h : h + 1],
                sacc[:, 2 * h : 2 * h + 1],
                sacc[:, 2 * h + 1 : 2 * h + 2],
                op=mybir.AluOpType.add,
            )
        nc.vector.reciprocal(rec[:], rec[:])
        for h in range(PPC):
            nc.vector.tensor_scalar_mul(
                probs16[:, K * h : K * (h + 1)],
                exp32[:, K * h : K * (h + 1)],
                rec[:, h : h + 1],
            )

        # ---- build the batch-interleaved gather table ----
        # TBL[ch, m, j] = probs16[(ch % 16) + 16*j, m]
        TBL = const.tile([128, NLOC, 4], _F16)
        for j in range(4):
            nc.sync.dma_start(
                TBL[0:16, :, j : j + 1],
                probs16[16 * j : 16 * (j + 1), :].unsqueeze(2),
            )
        # replicate partitions 0-15 to all 8 core groups (tree doubling)
        nc.sync.dma_start(TBL[16:32, :, :], TBL[0:16, :, :])
        nc.sync.dma_start(TBL[32:64, :, :], TBL[0:32, :, :])
        nc.sync.dma_start(TBL[64:128, :, :], TBL[0:64, :, :])

        # ---- gather + partitioning-sum + store ----
        # chunk t: Q7 group g handles classes [CG*g + JCC*t, CG*g + JCC*(t+1))
        out_ap = out_d.ap().rearrange("(j p) c -> p j c", j=4)
        for t in range(NCH):
            g = gpool.tile([128, NIDX, 4], _F16, tag="g")
            nc.gpsimd.ap_gather(
                g[:, :, :],
                TBL[:, :, :],
                idx_sb[:, ICOLS * t : ICOLS * (t + 1)],
                channels=128,
                num_elems=NLOC,
                d=4,
                num_idxs=NIDX,
            )
            a = apool.tile([128, 4, JCC], _F16, tag="a")
            nc.vector.tensor_add(
                a[:],
                g[:, 0:JCC, :].transpose([0, 2, 1]),
                g[:, JCC : 2 * JCC, :].transpose([0, 2, 1]),
            )
            for q in range(NG):
                c0 = CG * q + JCC * t
                nc.sync.dma_start(
                    out_ap[:, :, c0 : c0 + JCC],
                    a[16 * q : 16 * (q + 1), :, :],
                )

    nc.compile()
    return nc


def _host_inputs(x, W, b, part):
    """Per-core input maps: xT, wtb (W.T shard + bias row), gather indices."""
    xT = np.ascontiguousarray(x.T.astype(np.float16))
    part = np.asarray(part).astype(np.int64, copy=False)
    in_maps = []
    for i in range(NCORES):
        r0 = NLOC * i
        wtb = np.empty((D + 1, NLOC), np.float16)
        wtb[:D] = W[r0 : r0 + NLOC].T
        wtb[D] = b[r0 : r0 + NLOC]

        pa = (part[2 * i] - (2 * i) * K).astype(np.int16)        # [C] in [0, K)
        pb = (part[2 * i + 1] - (2 * i + 1) * K + K).astype(np.int16)  # [K, 2K)
        idxh = np.zeros((128, ICT), np.int16)
        for q in range(NG):
            for t in range(NCH):
                c0 = CG * q + JCC * t
                s = np.zeros(ICOLS * 16, np.int16)
                s[0:JCC] = pa[c0 : c0 + JCC]
                s[JCC:NIDX] = pb[c0 : c0 + JCC]
                blk = s.reshape(ICOLS, 16).T  # pos i <- (partition i%16, col i//16)
                idxh[16 * q : 16 * (q + 1), ICOLS * t : ICOLS * (t + 1)] = blk
        in_maps.append({"xT": xT, "wtb": wtb, "idx": idxh})
    return in_maps


def kernel(**inputs):
    global LAST_RESULTS
    x = np.asarray(inputs["input"], dtype=np.float32)
    W = np.asarray(inputs["W"], dtype=np.float32)
    b = np.asarray(inputs["b"], dtype=np.float32)
    part = np.asarray(inputs["partitionings"])
    assert x.shape == (B, D) and W.shape == (P * K, D)

    if "nc" not in _CACHE:
        _CACHE["nc"] = _build_nc()
    nc = _CACHE["nc"]

    in_maps = _host_inputs(x, W, b, part)
    trace = bool(int(os.environ.get("BASSK_TRACE", "0")))
    res = bass_utils.run_bass_kernel_spmd(
        nc,
        in_maps,
        core_ids=list(range(NCORES)),
        trace=trace,
        tmpdir=os.environ.get("BASSK_TRACE_DIR") or None,
    )
    LAST_RESULTS = res

    acc = res.results[0]["part_out"].astype(np.float32)
    for i in range(1, NCORES):
        acc = acc + res.results[i]["part_out"].astype(np.float32)
    tot = acc.sum(axis=1, keepdims=True)
    return np.log(acc / tot + ESP).astype(np.float32)


# revision 3
# speedup vs baseline: 3.0190x; 1.5726x over previous
"""Trainium2 Bass kernel for nn_CombinatorialClassifier (segment_reduce).

Strategy (8 NeuronCores, tensor-parallel over the num_partitionings axis):
  - Core i owns partitionings {2i, 2i+1}: a [2000, 2048] slice of W.
  - On device: logits = x @ Wshard.T (+ b row folded into the matmul),
    per-partitioning softmax -> probs [64, 2000] (fp16).
  - Gather: probs are repacked into a batch-interleaved table
    TBL[ch, m, j] = probs[(ch%16) + 16*j, m] (fp16, d=4), replicated
    across all 8 Q7 core groups. One ap_gather index then fetches the
    probs for FOUR batch rows at once, so each Q7 core only processes
    2*6250 indices (its own 6250-class range, both partitionings)
    instead of 50000 -- ~3.3x less gpsimd command overhead than the
    d=1 layout.
  - The two partitionings are summed on the vector engine (with a free
    (class, j) -> (j, class) transpose fused in), then DMAed to DRAM as
    a fp16 [64, 50000] partial.
  - Host: sum the 8 partials over cores (the all-reduce of the sharding
    hint), normalize over classes, log.
"""

import os
from contextlib import ExitStack

import numpy as np

import concourse.bacc as bacc
import concourse.mybir as mybir
import concourse.tile as tile
from concourse import bass_utils
from concourse import library_config

B, P, K, C, D = 64, 16, 1000, 50000, 2048
ESP = 1e-20
NCORES = 8
PPC = P // NCORES        # partitionings per core (2)
NLOC = PPC * K           # local logits width (2000)
NT = 500                 # matmul N-tile (PSUM bank: 500 fp32 <= 512)
NNT = NLOC // NT         # 4 N-tiles
DCH = D // 128           # 16 contraction chunks of 128

NG = 8                   # Q7 core groups (16 partitions each)
CG = C // NG             # classes per group (6250)
NCH = 5                  # gather chunks per group
JCC = CG // NCH          # classes per chunk (1250)
NIDX = 2 * JCC           # gather indices per chunk (A then B) = 2500
ICOLS = ((NIDX + 31) // 32) * 2  # int16 idx cols per chunk (158)
ICT = NCH * ICOLS        # total idx cols (790)

_F32 = mybir.dt.float32
_F16 = mybir.dt.float16
_I16 = mybir.dt.int16

_CACHE = {}
LAST_RESULTS = None


def _build_nc():
    nc = bacc.Bacc(
        "TRN2",
        target_bir_lowering=False,
        debug=False,
        enable_asserts=False,
        num_devices=NCORES,
    )
    xT_d = nc.dram_tensor("xT", [D, B], _F16, kind="ExternalInput")
    wtb_d = nc.dram_tensor("wtb", [D + 1, NLOC], _F16, kind="ExternalInput")
    idx_d = nc.dram_tensor("idx", [128, ICT], _I16, kind="ExternalInput")
    out_d = nc.dram_tensor("part_out", [B, C], _F16, kind="ExternalOutput")

    with tile.TileContext(nc) as tc, ExitStack() as ctx:
        const = ctx.enter_context(tc.tile_pool(name="const", bufs=1))
        wpool = ctx.enter_context(tc.tile_pool(name="w", bufs=3))
        spool = ctx.enter_context(tc.tile_pool(name="stats", bufs=1))
        gpool = ctx.enter_context(tc.tile_pool(name="g", bufs=3))
        apool = ctx.enter_context(tc.tile_pool(name="a", bufs=3))
        psum = ctx.enter_context(
            tc.tile_pool(name="psum", bufs=1, space="PSUM")
        )

        xt = const.tile([128, DCH, B], _F16)
        nc.sync.dma_start(xt[:], xT_d.ap().rearrange("(c p) b -> p c b", p=128))
        ones = const.tile([1, B], _F16)
        nc.vector.memset(ones[:], 1.0)
        bias = const.tile([1, NLOC], _F16)
        nc.sync.dma_start(bias[:], wtb_d[D : D + 1, :])
        idx_sb = const.tile([128, ICT], _I16)
        nc.sync.dma_start(idx_sb[:], idx_d.ap())

        # get the gpsimd library load off the critical path: it can load
        # during the matmul instead of right before the first gather
        nc.gpsimd.load_library(library_config.ap_gather)

        # ---- logits = x @ Wshard.T + b, accumulated in 4 PSUM banks ----
        ps = [psum.tile([B, NT], _F32, tag=f"ps{n}", name=f"ps{n}") for n in range(NNT)]
        for j in range(DCH):
            wt = wpool.tile([128, NLOC], _F16)
            nc.sync.dma_start(wt[:], wtb_d[128 * j : 128 * (j + 1), :])
            for n in range(NNT):
                nc.tensor.matmul(
                    ps[n][:],
                    xt[:, j, :],
                    wt[:, NT * n : NT * (n + 1)],
                    start=(j == 0),
                    stop=False,
                )
        for n in range(NNT):
            nc.tensor.matmul(
                ps[n][:],
                ones[:],
                bias[:, NT * n : NT * (n + 1)],
                start=False,
                stop=True,
            )

        # ---- per-partitioning softmax -> probs16 [64, 2000] fp16 ----
        exp32 = const.tile([B, NLOC], _F32)
        probs16 = const.tile([B, NLOC], _F16)
        mx = spool.tile([B, NNT], _F32)
        for n in range(NNT):
            nc.vector.reduce_max(
                mx[:, n : n + 1], ps[n][:], axis=mybir.AxisListType.X
            )
        neg = spool.tile([B, PPC], _F32)
        for h in range(PPC):
            nc.vector.tensor_tensor(
                neg[:, h : h + 1],
                mx[:, 2 * h : 2 * h + 1],
                mx[:, 2 * h + 1 : 2 * h + 2],
                op=mybir.AluOpType.max,
            )
        nc.vector.tensor_scalar_mul(neg[:], neg[:], -1.0)
        sacc = spool.tile([B, NNT], _F32)
        for n in range(NNT):
            h = n // 2
            nc.scalar.activation(
                exp32[:, NT * n : NT * (n + 1)],
                ps[n][:],
                mybir.ActivationFunctionType.Exp,
                bias=neg[:, h : h + 1],
                accum_out=sacc[:, n : n + 1],
            )
        rec = spool.tile([B, PPC], _F32)
        for h in range(PPC):
            nc.vector.tensor_tensor(
                rec[:, h : h + 1],
                sacc[:, 2 * h : 2 * h + 1],
                sacc[:, 2 * h + 1 : 2 * h + 2],
                op=mybir.AluOpType.add,
            )
        nc.vector.reciprocal(rec[:], rec[:])
        for h in range(PPC):
            nc.vector.tensor_scalar_mul(
                probs16[:, K * h : K * (h + 1)],
                exp32[:, K * h : K * (h + 1)],
                rec[:, h : h + 1],
            )

        # ---- build the batch-interleaved gather table ----
        # TBL[ch, m, j] = probs16[(ch % 16) + 16*j, m]
        # stage[ch, j, m] first (contiguous DMAs + tree broadcast), then a
        # single strided vector copy does the (j, m) -> (m, j) interleave;
        # a strided DMA here would lower to a catastrophically slow
        # 2-byte-element DMA_DIRECT2D.
        stage = const.tile([128, 4, NLOC], _F16)
        for j in range(4):
            nc.sync.dma_start(
                stage[0:16, j : j + 1, :],
                probs16[16 * j : 16 * (j + 1), :].unsqueeze(1),
            )
        nc.sync.dma_start(stage[16:32, :, :], stage[0:16, :, :])
        nc.sync.dma_start(stage[32:64, :, :], stage[0:32, :, :])
        nc.sync.dma_start(stage[64:128, :, :], stage[0:64, :, :])
        TBL = const.tile([128, NLOC, 4], _F16)
        nc.vector.tensor_copy(TBL[:, :, :].transpose([0, 2, 1]), stage[:, :, :])

        # ---- gather + partitioning-sum + store ----
        # chunk t: Q7 group g handles classes [CG*g + JCC*t, CG*g + JCC*(t+1))
        out_ap = out_d.ap().rearrange("(j p) c -> p j c", j=4)
        for t in range(NCH):
            g = gpool.tile([128, NIDX, 4], _F16, tag="g")
            nc.gpsimd.ap_gather(
                g[:, :, :],
                TBL[:, :, :],
                idx_sb[:, ICOLS * t : ICOLS * (t + 1)],
                channels=128,
                num_elems=NLOC,
                d=4,
                num_idxs=NIDX,
            )
            a = apool.tile([128, 4, JCC], _F16, tag="a")
            nc.vector.tensor_add(
                a[:],
                g[:, 0:JCC, :].transpose([0, 2, 1]),
                g[:, JCC : 2 * JCC, :].transpose([0, 2, 1]),
            )
            for q in range(NG):
                c0 = CG * q + JCC * t
                nc.sync.dma_start(
                    out_ap[:, :, c0 : c0 + JCC],
                    a[16 * q : 16 * (q + 1), :, :],
                )

    nc.compile()
    return nc


def _host_inputs(x, W, b, part):
    """Per-core input maps: xT, wtb (W.T shard + bias row), gather indices."""
    xT = np.ascontiguousarray(x.T.astype(np.float16))
    part = np.asarray(part).astype(np.int64, copy=False)
    in_maps = []
    for i in range(NCORES):
        r0 = NLOC * i
        wtb = np.empty((D + 1, NLOC), np.float16)
        wtb[:D] = W[r0 : r0 + NLOC].T
        wtb[D] = b[r0 : r0 + NLOC]

        pa = (part[2 * i] - (2 * i) * K).astype(np.int16)        # [C] in [0, K)
        pb = (part[2 * i + 1] - (2 * i + 1) * K + K).astype(np.int16)  # [K, 2K)
        idxh = np.zeros((128, ICT), np.int16)
        for q in range(NG):
            for t in range(NCH):
                c0 = CG * q + JCC * t
                s = np.zeros(ICOLS * 16, np.int16)
                s[0:JCC] = pa[c0 : c0 + JCC]
                s[JCC:NIDX] = pb[c0 : c0 + JCC]
                blk = s.reshape(ICOLS, 16).T  # pos i <- (partition i%16, col i//16)
                idxh[16 * q : 16 * (q + 1), ICOLS * t : ICOLS * (t + 1)] = blk
        in_maps.append({"xT": xT, "wtb": wtb, "idx": idxh})
    return in_maps


def kernel(**inputs):
    global LAST_RESULTS
    x = np.asarray(inputs["input"], dtype=np.float32)
    W = np.asarray(inputs["W"], dtype=np.float32)
    b = np.asarray(inputs["b"], dtype=np.float32)
    part = np.asarray(inputs["partitionings"])
    assert x.shape == (B, D) and W.shape == (P * K, D)

    if "nc" not in _CACHE:
        _CACHE["nc"] = _build_nc()
    nc = _CACHE["nc"]

    in_maps = _host_inputs(x, W, b, part)
    trace = bool(int(os.environ.get("BASSK_TRACE", "0")))
    res = bass_utils.run_bass_kernel_spmd(
        nc,
        in_maps,
        core_ids=list(range(NCORES)),
        trace=trace,
        tmpdir=os.environ.get("BASSK_TRACE_DIR") or None,
    )
    LAST_RESULTS = res

    acc = res.results[0]["part_out"].astype(np.float32)
    for i in range(1, NCORES):
        acc = acc + res.results[i]["part_out"].astype(np.float32)
    tot = acc.sum(axis=1, keepdims=True)
    return np.log(acc / tot + ESP).astype(np.float32)


# revision 4
# speedup vs baseline: 3.1177x; 1.0327x over previous
"""Trainium2 Bass kernel for nn_CombinatorialClassifier (segment_reduce).

Strategy (8 NeuronCores, tensor-parallel over the num_partitionings axis):
  - Core i owns partitionings {2i, 2i+1} (= A, B): a [2000, 2048] slice
    of W.
  - On device, per partitioning h in (A, B): logits_h = x @ W_h.T (+ b
    row folded into the matmul), softmax -> probs_h [64, 1000] fp16,
    repacked into a batch-interleaved gather table
    TBL_h[ch, m, j] = probs_h[(ch%16) + 16*j, m] (d=4) replicated
    across all 8 Q7 core groups.  The A pipeline runs first so A
    gathers start while B is still in matmul/softmax.
  - Gather: one ap_gather index fetches the probs for FOUR batch rows
    at once, and each Q7 core group owns its own 6250-class range, so
    each Q7 core processes 2*6250 indices instead of 50000 -- ~3.3x
    less gpsimd command overhead than the d=1 layout.  A and B chunks
    alternate; after each (A, B) pair the vector engine sums them (with
    a (class, j) -> (j, class) transpose fused in) and the fp16
    [64, 50000] partial is DMAed out.
  - Host: sum the 8 partials over cores (the all-reduce of the sharding
    hint), normalize over classes, log.
"""

import os
from contextlib import ExitStack

import numpy as np

import concourse.bacc as bacc
import concourse.mybir as mybir
import concourse.tile as tile
from concourse import bass_utils
from concourse import library_config

B, P, K, C, D = 64, 16, 1000, 50000, 2048
ESP = 1e-20
NCORES = 8
PPC = P // NCORES        # partitionings per core (2)
NLOC = PPC * K           # local logits width (2000)
NT = 500                 # matmul N-tile (PSUM bank: 500 fp32 <= 512)
DCH = D // 128           # 16 contraction chunks of 128

NG = 8                   # Q7 core groups (16 partitions each)
CG = C // NG             # classes per group (6250)
NCH = 5                  # gather chunks per group per partitioning
JCC = CG // NCH          # classes per chunk (1250)
NIDX = ((JCC + 3) // 4) * 4          # padded gather indices per call (1252)
ICOLS = ((NIDX + 31) // 32) * 2      # int16 idx cols per call (80)
ICT = 2 * NCH * ICOLS                # total idx cols (800)

_F32 = mybir.dt.float32
_F16 = mybir.dt.float16
_I16 = mybir.dt.int16

_CACHE = {}
LAST_RESULTS = None


def _build_nc():
    nc = bacc.Bacc(
        "TRN2",
        target_bir_lowering=False,
        debug=False,
        enable_asserts=False,
        num_devices=NCORES,
    )
    xT_d = nc.dram_tensor("xT", [D, B], _F16, kind="ExternalInput")
    wtb_d = nc.dram_tensor("wtb", [D + 1, NLOC], _F16, kind="ExternalInput")
    idx_d = nc.dram_tensor("idx", [128, ICT], _I16, kind="ExternalInput")
    out_d = nc.dram_tensor("part_out", [B, C], _F16, kind="ExternalOutput")

    with tile.TileContext(nc) as tc, ExitStack() as ctx:
        const = ctx.enter_context(tc.tile_pool(name="const", bufs=1))
        wpool = ctx.enter_context(tc.tile_pool(name="w", bufs=4))
        spool = ctx.enter_context(tc.tile_pool(name="stats", bufs=1))
        gpool = ctx.enter_context(tc.tile_pool(name="g", bufs=4))
        apool = ctx.enter_context(tc.tile_pool(name="a", bufs=3))
        psum = ctx.enter_context(
            tc.tile_pool(name="psum", bufs=1, space="PSUM")
        )

        xt = const.tile([128, DCH, B], _F16)
        nc.sync.dma_start(xt[:], xT_d.ap().rearrange("(c p) b -> p c b", p=128))
        ones = const.tile([1, B], _F16)
        nc.vector.memset(ones[:], 1.0)
        bias = const.tile([1, NLOC], _F16)
        nc.sync.dma_start(bias[:], wtb_d[D : D + 1, :])
        idx_sb = const.tile([128, ICT], _I16)
        nc.sync.dma_start(idx_sb[:], idx_d.ap())

        # get the gpsimd library load off the critical path: it can load
        # during the matmul instead of right before the first gather
        nc.gpsimd.load_library(library_config.ap_gather)

        ps = [psum.tile([B, NT], _F32, tag=f"ps{n}", name=f"ps{n}") for n in range(4)]
        TBLs = []
        # ---- per-partitioning pipeline: matmul -> softmax -> table ----
        # A (h=0) runs fully first so its gathers can start while B is
        # still in flight on the tensor/vector/scalar engines.
        for h in range(PPC):
            k0 = K * h
            for j in range(DCH):
                wt = wpool.tile([128, K], _F16, tag="w")
                nc.sync.dma_start(
                    wt[:], wtb_d[128 * j : 128 * (j + 1), k0 : k0 + K]
                )
                for n in range(2):
                    nc.tensor.matmul(
                        ps[2 * h + n][:],
                        xt[:, j, :],
                        wt[:, NT * n : NT * (n + 1)],
                        start=(j == 0),
                        stop=False,
                    )
            for n in range(2):
                nc.tensor.matmul(
                    ps[2 * h + n][:],
                    ones[:],
                    bias[:, k0 + NT * n : k0 + NT * (n + 1)],
                    start=False,
                    stop=True,
                )

            # softmax over the two PSUM banks of this partitioning
            mx = spool.tile([B, 2], _F32, tag=f"mx{h}")
            for n in range(2):
                nc.vector.reduce_max(
                    mx[:, n : n + 1], ps[2 * h + n][:], axis=mybir.AxisListType.X
                )
            neg = spool.tile([B, 1], _F32, tag=f"neg{h}")
            nc.vector.tensor_tensor(
                neg[:], mx[:, 0:1], mx[:, 1:2], op=mybir.AluOpType.max
            )
            nc.vector.tensor_scalar_mul(neg[:], neg[:], -1.0)
            exp32 = spool.tile([B, K], _F32, tag=f"exp{h}")
            sacc = spool.tile([B, 2], _F32, tag=f"sacc{h}")
            for n in range(2):
                nc.scalar.activation(
                    exp32[:, NT * n : NT * (n + 1)],
                    ps[2 * h + n][:],
                    mybir.ActivationFunctionType.Exp,
                    bias=neg[:],
                    accum_out=sacc[:, n : n + 1],
                )
            rec = spool.tile([B, 1], _F32, tag=f"rec{h}")
            nc.vector.tensor_tensor(
                rec[:], sacc[:, 0:1], sacc[:, 1:2], op=mybir.AluOpType.add
            )
            nc.vector.reciprocal(rec[:], rec[:])
            probs16 = spool.tile([B, K], _F16, tag=f"p16{h}")
            nc.vector.tensor_scalar_mul(probs16[:], exp32[:], rec[:])

            # batch-interleaved gather table:
            # TBL[ch, m, j] = probs16[(ch % 16) + 16*j, m].
            # stage[ch, j, m] first (contiguous DMAs + tree broadcast),
            # then one strided vector copy does the (j, m) -> (m, j)
            # interleave; a strided DMA here would lower to a
            # catastrophically slow 2-byte-element DMA_DIRECT2D.
            stage = spool.tile([128, 4, K], _F16, tag=f"st{h}")
            for j in range(4):
                nc.sync.dma_start(
                    stage[0:16, j : j + 1, :],
                    probs16[16 * j : 16 * (j + 1), :].unsqueeze(1),
                )
            nc.sync.dma_start(stage[16:32, :, :], stage[0:16, :, :])
            nc.sync.dma_start(stage[32:64, :, :], stage[0:32, :, :])
            nc.sync.dma_start(stage[64:128, :, :], stage[0:64, :, :])
            TBL = spool.tile([128, K, 4], _F16, tag=f"tbl{h}")
            nc.vector.tensor_copy(
                TBL[:, :, :].transpose([0, 2, 1]), stage[:, :, :]
            )
            TBLs.append(TBL)

        # ---- gather + partitioning-sum + store ----
        # chunk t: Q7 group q handles classes [CG*q + JCC*t, CG*q + JCC*(t+1))
        out_ap = out_d.ap().rearrange("(j p) c -> p j c", j=4)
        for t in range(NCH):
            gs = []
            for h in range(PPC):
                g = gpool.tile([128, NIDX, 4], _F16, tag="g")
                nc.gpsimd.ap_gather(
                    g[:, :, :],
                    TBLs[h][:, :, :],
                    idx_sb[:, ICOLS * (NCH * h + t) : ICOLS * (NCH * h + t + 1)],
                    channels=128,
                    num_elems=K,
                    d=4,
                    num_idxs=NIDX,
                )
                gs.append(g)
            a = apool.tile([128, 4, JCC], _F16, tag="a")
            nc.vector.tensor_add(
                a[:],
                gs[0][:, 0:JCC, :].transpose([0, 2, 1]),
                gs[1][:, 0:JCC, :].transpose([0, 2, 1]),
            )
            for q in range(NG):
                c0 = CG * q + JCC * t
                nc.sync.dma_start(
                    out_ap[:, :, c0 : c0 + JCC],
                    a[16 * q : 16 * (q + 1), :, :],
                )

    nc.compile()
    return nc


def _host_inputs(x, W, b, part):
    """Per-core input maps: xT, wtb (W.T shard + bias row), gather indices."""
    xT = np.ascontiguousarray(x.T.astype(np.float16))
    part = np.asarray(part).astype(np.int64, copy=False)
    in_maps = []
    for i in range(NCORES):
        r0 = NLOC * i
        wtb = np.empty((D + 1, NLOC), np.float16)
        wtb[:D] = W[r0 : r0 + NLOC].T
        wtb[D] = b[r0 : r0 + NLOC]

        idxh = np.zeros((128, ICT), np.int16)
        for h in range(PPC):
            ph = (part[2 * i + h] - (2 * i + h) * K).astype(np.int16)  # [C] in [0, K)
            for q in range(NG):
                for t in range(NCH):
                    c0 = CG * q + JCC * t
                    s = np.zeros(ICOLS * 16, np.int16)
                    s[0:JCC] = ph[c0 : c0 + JCC]
                    blk = s.reshape(ICOLS, 16).T  # pos i <- (partition i%16, col i//16)
                    col = ICOLS * (NCH * h + t)
                    idxh[16 * q : 16 * (q + 1), col : col + ICOLS] = blk
        in_maps.append({"xT": xT, "wtb": wtb, "idx": idxh})
    return in_maps


def kernel(**inputs):
    global LAST_RESULTS
    x = np.asarray(inputs["input"], dtype=np.float32)
    W = np.asarray(inputs["W"], dtype=np.float32)
    b = np.asarray(inputs["b"], dtype=np.float32)
    part = np.asarray(inputs["partitionings"])
    assert x.shape == (B, D) and W.shape == (P * K, D)

    if "nc" not in _CACHE:
        _CACHE["nc"] = _build_nc()
    nc = _CACHE["nc"]

    in_maps = _host_inputs(x, W, b, part)
    trace = bool(int(os.environ.get("BASSK_TRACE", "0")))
    res = bass_utils.run_bass_kernel_spmd(
        nc,
        in_maps,
        core_ids=list(range(NCORES)),
        trace=trace,
        tmpdir=os.environ.get("BASSK_TRACE_DIR") or None,
    )
    LAST_RESULTS = res

    acc = res.results[0]["part_out"].astype(np.float32)
    for i in range(1, NCORES):
        acc = acc + res.results[i]["part_out"].astype(np.float32)
    tot = acc.sum(axis=1, keepdims=True)
    return np.log(acc / tot + ESP).astype(np.float32)
